# revision 23
# baseline (speedup 1.0000x reference)
"""BitNet MNIST MLP forward on 8 Trainium2 NeuronCores (pure data parallel).

Reference math (per _bitlinear): out = (x/sx) @ w_q.T * sx with per-row
sx = max(|x|) -- the activation scale cancels exactly, so we compute
x @ w_q.T directly.  Ternary w_q is precomputed on host (exact in bf16).

Per-core dataflow (batch shard 8192 rows, chunks of 512 batch columns):
  activations live feature-major [feat_part(128) x batch_free] in SBUF, so
  every layer's matmul contracts features on partitions with stationary
  (pre-transposed) weights and NO on-chip transposes.
  RMS mean(h^2) over the 1024 features = ones(1/1024)-matmul accumulated
  over the 8 feature tiles -> replicated [128, 512] PSUM value.
  rsqrt = int bit-trick seed + 1 Newton iteration on DVE (no ACT table
  thrash; ScalarE only runs {copy, gelu} = one table set).

v2 scheduling: the per-layer stats matmul (ssq) is deferred into the
MIDDLE of the NEXT layer's matmul stream so the PE never stalls on the
DVE square/tree chain.  PE emission order per superstep s:
  [L1(s) 56MM] [ssq2(s-2)] [L2(s-1) 64MM] [ssq1(s)] [L3(s-2) 8MM packed]
PSUM mm tiles are 2-bank [128,2,512] so PSUM->SBUF evacuation is 4 ACT
copies per layer-chunk instead of 8.  L3 emits its 4 col-group MMs
back-to-back per k-step so they run concurrently (tile_position packing).
"""

import os
from contextlib import ExitStack

import numpy as np
import ml_dtypes

import concourse.bacc as bacc
import concourse.bass as bass
import concourse.mybir as mybir
import concourse.tile as tile
from concourse.bass_utils import run_bass_kernel_spmd

N_CORES = 8
B, IN, H, OUT = 65536, 784, 1024, 10
BPC = B // N_CORES  # 8192 rows per core
KP = 896            # 784 zero-padded to 7*128
K1 = KP // 128      # 7 contraction tiles, layer 1
K2 = H // 128       # 8 contraction tiles, layers 2/3
HO = H // 128       # 8 output-feature tiles
BS = 512            # batch columns per chunk
NB = BPC // BS      # 16 chunks
EPS_Q = 1e-5
MAGIC = 0x5F3759DF

F32 = mybir.dt.float32
BF16 = mybir.dt.bfloat16
I32 = mybir.dt.int32
ALU = mybir.AluOpType
ACTF = mybir.ActivationFunctionType

_cache = {}
LAST_RESULTS = None  # test.py reads exec_time_ns off this


def _build(g_is_one=True):
    # Bacc (not raw Bass): its compile() runs generate_event_semaphores(),
    # which splits multi-wait sync_infos down to the 1-wait HW limit.
    nc = bacc.Bacc("TRN2", target_bir_lowering=False, debug=False, num_devices=N_CORES)

    xt = nc.dram_tensor("xt", [KP, BPC], BF16, kind="ExternalInput").ap()
    w1t = nc.dram_tensor("w1t", [KP, H], BF16, kind="ExternalInput").ap()
    w2t = nc.dram_tensor("w2t", [H, H], BF16, kind="ExternalInput").ap()
    w3t = nc.dram_tensor("w3t", [H, OUT], BF16, kind="ExternalInput").ap()
    g1 = nc.dram_tensor("g1", [128, HO], F32, kind="ExternalInput").ap()
    g2 = nc.dram_tensor("g2", [128, HO], F32, kind="ExternalInput").ap()
    outt = nc.dram_tensor("outt", [OUT, BPC], F32, kind="ExternalOutput").ap()

    with tile.TileContext(nc) as tc, ExitStack() as ctx:
        wp = ctx.enter_context(tc.tile_pool(name="weights", bufs=1))
        xp = ctx.enter_context(tc.tile_pool(name="x", bufs=3))
        hp = ctx.enter_context(tc.tile_pool(name="h", bufs=2))       # hraw1/hraw2 (intra-superstep)
        sq = ctx.enter_context(tc.tile_pool(name="sq", bufs=2))      # hsq/hs scratch (short-lived)
        tp = ctx.enter_context(tc.tile_pool(name="tree", bufs=2))    # pairs/quads/octs
        hq = ctx.enter_context(tc.tile_pool(name="hout", bufs=5))    # gelu outs (cross superstep)
        rp = ctx.enter_context(tc.tile_pool(name="rsq", bufs=2))
        op = ctx.enter_context(tc.tile_pool(name="out", bufs=3))
        pp = ctx.enter_context(tc.tile_pool(name="ps", bufs=2, space="PSUM"))   # 2x [128,2,512]
        sp = ctx.enter_context(tc.tile_pool(name="ssq", bufs=2, space="PSUM"))  # 2x [128,512]
        p3 = ctx.enter_context(tc.tile_pool(name="ps3", bufs=2, space="PSUM"))  # 2x [128,512]

        # --- resident weights; interleave w1[k] with x0[k] so the first L1
        # matmuls can start as soon as their own k-slice pair lands ---
        w1sb = wp.tile([128, K1, H], BF16)
        x0sb = xp.tile([128, K1, BS], BF16, tag="xsb")
        xt_r0 = xt.rearrange("(k p) b -> p k b", p=128)
        for k in range(K1):
            nc.sync.dma_start(w1sb[:, k, :], w1t[k * 128 : (k + 1) * 128, :])
            nc.sync.dma_start(x0sb[:, k, :], xt_r0[:, k, 0:BS])
        g1sb = wp.tile([128, HO], F32)
        nc.sync.dma_start(g1sb[:], g1[:])
        ones = wp.tile([128, 128], BF16)
        # 0.5/H: ssq = mean(h^2)/2 so the rsqrt Newton step needs no 0.5 factor
        nc.vector.memset(ones[:], 0.5 / H)
        # layer-2/3 weights aren't needed until superstep 1 -> emit after so
        # the HWDGE lanes serve x-chunk-0 + w1 first
        w2sb = wp.tile([128, K2, H], BF16)
        w3sb = wp.tile([128, K2, OUT], BF16)
        g2sb = wp.tile([128, HO], F32)

        def load_l23_weights():
            for k in range(K2):
                nc.sync.dma_start(w2sb[:, k, :], w2t[k * 128 : (k + 1) * 128, :])
            for k in range(K2):
                nc.sync.dma_start(w3sb[:, k, :], w3t[k * 128 : (k + 1) * 128, :])
            nc.sync.dma_start(g2sb[:], g2[:])

        xt_r = xt.rearrange("(k p) b -> p k b", p=128)

        def load_x(s):
            xsb = xp.tile([128, K1, BS], BF16, tag="xsb")
            bsl = slice(s * BS, (s + 1) * BS)
            for k in range(K1):
                nc.sync.dma_start(xsb[:, k, :], xt_r[:, k, bsl])
            return xsb

        def mm_phase(rhs, w_sb, nk, tag, defer_stats=False, mid_emit=None):
            """PE: h = rhs.T @ W as 4 pair-groups -> hraw [128, HO, BS] bf16
            via 2-bank PSUM tiles + 4 two-bank ACT copies.  DVE: square +
            pairwise reduce tree down to octs [128, BS].  With defer_stats
            the DVE part is returned as a thunk so the caller can emit it
            AFTER a latency-critical norm chain in the DVE queue.  mid_emit
            (if given) is called after pair-group 1 so a dependent norm
            chain can start mid-stream (used for the pipeline drain)."""
            hraw = hp.tile([128, HO, BS], BF16, tag=f"hraw{tag}")
            for j in range(HO // 2):
                ps = pp.tile([128, 2, BS], F32, tag="mm")
                for jj in range(2):
                    oi = 2 * j + jj
                    for k in range(nk):
                        nc.tensor.matmul(
                            ps[:, jj, :],
                            lhsT=w_sb[:, k, oi * 128 : (oi + 1) * 128],
                            rhs=rhs[:, k, :],
                            start=(k == 0),
                            stop=(k == nk - 1),
                        )
                nc.scalar.copy(hraw[:, 2 * j : 2 * j + 2, :], ps[:])
                if j == 0 and mid_emit is not None:
                    mid_emit()

            def stats():
                hsq = sq.tile([128, HO, BS], BF16, tag="hsq")
                for j in range(HO // 2):
                    # per-pair squares start as soon as each ACT copy lands
                    nc.vector.tensor_mul(
                        hsq[:, 2 * j : 2 * j + 2, :],
                        hraw[:, 2 * j : 2 * j + 2, :],
                        hraw[:, 2 * j : 2 * j + 2, :],
                    )
                pairs = tp.tile([128, HO // 2, BS], BF16, tag="prs")
                ev = hsq[:].rearrange("p (j two) f -> p two j f", two=2)
                nc.vector.tensor_add(pairs[:], ev[:, 0], ev[:, 1])
                quads = tp.tile([128, 2, BS], BF16, tag="qds")
                nc.vector.tensor_add(quads[:], pairs[:, 0:2, :], pairs[:, 2:4, :])
                octs = tp.tile([128, BS], BF16, tag=f"oct{tag}")
                nc.vector.tensor_add(octs[:], quads[:, 0, :], quads[:, 1, :])
                return octs

            if defer_stats:
                return hraw, stats
            return hraw, stats()

        def ssq_mm(octs):
            """PE: ONE ones-matmul partition-reduce -> replicated [128,BS]."""
            ssq = sp.tile([128, BS], F32, tag="ssq")
            nc.tensor.matmul(ssq[:], lhsT=ones[:], rhs=octs[:], start=True, stop=True)
            return ssq

        def norm_phase(hraw, ssq, g_sb, fine=False):
            """DVE: rsqrt via magic seed + 1 Newton step, prescale.
            ACT: gelu -> hout [128, HO, BS]."""
            hs = sq.tile([128, HO, BS], BF16, tag="hs")
            hout = hq.tile([128, HO, BS], BF16, tag="hout")
            # rsqrt(2*ssq) fully on DVE in 4 ops: magic seed (constant shifted
            # by -0x400000 to absorb the ssq=mean/2 scale) + one fused
            # Newton step via the RECIPROCAL_APPROX_NR custom op
            # rinv = y0*(1.5 - ssq*y0^2),  y0 = bits(0x5F3359E0 + ~(i>>1))
            from concourse.dve_ops import RECIPROCAL_APPROX_NR

            ti = rp.tile([128, BS], I32, tag="ti")
            nc.vector.tensor_scalar(
                ti[:], ssq[:].bitcast(I32), 1, -1,
                op0=ALU.arith_shift_right, op1=ALU.bitwise_xor,
            )  # ~(v >> 1)
            nc.vector.tensor_scalar(
                ti[:], ti[:], MAGIC - 0x400000 + 1, None, op0=ALU.add
            )
            y0f = ti[:].bitcast(F32)
            u = rp.tile([128, BS], F32, tag="u")
            nc.vector.tensor_mul(u[:], ssq[:], y0f)
            rinv = rp.tile([128, BS], F32, tag="rinv")
            nc.vector._custom_dve(
                RECIPROCAL_APPROX_NR, out=rinv[:], in0=u[:], in1=y0f, s0=1.5
            )
            rb = rinv[:].rearrange("p (o f) -> p o f", o=1).broadcast_to([128, HO, BS])
            if fine:
                # emit in L3's consumption order (kk=0 strips read k=0,2,4,6)
                for oi in (0, 2, 4, 6, 1, 3, 5, 7):
                    nc.vector.tensor_mul(hs[:, oi, :], hraw[:, oi, :], rinv[:])
                    if g_is_one:
                        nc.scalar.activation(hout[:, oi, :], hs[:, oi, :], ACTF.Gelu)
                    else:
                        nc.scalar.activation(
                            hout[:, oi, :], hs[:, oi, :], ACTF.Gelu,
                            scale=g_sb[:, oi : oi + 1],
                        )
            elif g_is_one:
                # halves: the first half of hout lands ~3us earlier, giving
                # the next layer's first matmuls their rhs sooner
                hh = HO // 2
                nc.vector.tensor_mul(hs[:, :hh, :], hraw[:, :hh, :], rb[:, :hh, :])
                nc.scalar.activation(hout[:, :hh, :], hs[:, :hh, :], ACTF.Gelu)
                nc.vector.tensor_mul(hs[:, hh:, :], hraw[:, hh:, :], rb[:, hh:, :])
                nc.scalar.activation(hout[:, hh:, :], hs[:, hh:, :], ACTF.Gelu)
            else:
                nc.vector.tensor_mul(hs[:], hraw[:], rb)
                for oi in range(HO):
                    nc.scalar.activation(
                        hout[:, oi, :], hs[:, oi, :], ACTF.Gelu,
                        scale=g_sb[:, oi : oi + 1],
                    )
            return hout

        def l3_phase(h2, c):
            """L3 (M=10): 4 col-strips of the PE array; emit the 4 strips
            back-to-back per k-step so they run concurrently."""
            ps3 = p3.tile([128, BS], F32, tag="mm3")
            for kk in range(2):
                for g in range(4):
                    k = 2 * g + kk
                    nc.tensor.matmul(
                        ps3[32 * g : 32 * g + OUT, :],
                        lhsT=w3sb[:, k, :],
                        rhs=h2[:, k, :],
                        start=(kk == 0),
                        stop=(kk == 1),
                        tile_position=(0, 32 * g),
                    )
            osb = op.tile([OUT, BS], F32, tag="osb")
            nc.scalar.copy(osb[:], ps3[0:OUT, :])
            for g in range(1, 4):
                nc.vector.tensor_add(osb[:], osb[:], ps3[32 * g : 32 * g + OUT, :])
            nc.sync.dma_start(outt[:, c * BS : (c + 1) * BS], osb[:])

        # --- superstep pipeline ---
        # state: (hraw, octs) awaiting their ssq+norm
        st1: dict[int, object] = {}
        st2: dict[int, object] = {}
        h1s: dict[int, object] = {}
        h2s: dict[int, object] = {}
        xs: dict[int, object] = {}

        xs[0] = x0sb
        if NB > 1:
            xs[1] = load_x(1)

        for s in range(NB + 4):
            if s < NB:
                with nc.named_scope(f"L1_{s}"):
                    st1[s] = mm_phase(xs.pop(s), w1sb, K1, tag="1")
            if s + 2 < NB:
                with nc.named_scope(f"xdma_{s + 2}"):
                    xs[s + 2] = load_x(s + 2)
            if s == 0:
                load_l23_weights()
            if 2 <= s <= NB + 1:
                c = s - 2
                hraw2, octs2 = st2.pop(c)
                fine = c == NB - 1  # last chunk: minimize norm->L3 latency
                with nc.named_scope(f"n2_{c}"):
                    h2s[c] = norm_phase(hraw2, ssq_mm(octs2), g2sb, fine=fine)
            stats2 = None
            if 1 <= s <= NB:
                mid = None
                if s == NB - 1:
                    # last L1 chunk: emit its ssq+norm in the MIDDLE of this
                    # L2 stream so the final L2 (next superstep, which has no
                    # L1 cover) finds its rhs ready instead of stalling ~8us
                    hraw1m, octs1m = st1.pop(s)

                    def mid():
                        with nc.named_scope(f"n1_{s}"):
                            h1s[s] = norm_phase(hraw1m, ssq_mm(octs1m), g1sb)

                with nc.named_scope(f"L2_{s - 1}"):
                    hraw2n, stats2 = mm_phase(
                        h1s.pop(s - 1), w2sb, K2, tag="2",
                        defer_stats=True, mid_emit=mid,
                    )
            if s < NB and s != NB - 1:
                hraw1, octs1 = st1.pop(s)
                with nc.named_scope(f"n1_{s}"):
                    h1s[s] = norm_phase(hraw1, ssq_mm(octs1), g1sb)
            if stats2 is not None:
                # L2 stats DVE ops go AFTER norm1 in the DVE queue so the
                # latency-critical rsqrt->hs->gelu chain isn't queued behind
                # the (slack-rich) square/tree of L2
                with nc.named_scope(f"t2_{s - 1}"):
                    st2[s - 1] = (hraw2n, stats2())
            if 4 <= s <= NB + 3:
                # L3 two supersteps later than its norm: its gelu always has
                # a full superstep of cover, and the tail supersteps keep
                # ready L3 work to fill the drain gaps
                with nc.named_scope(f"L3_{s - 4}"):
                    l3_phase(h2s.pop(s - 4), s - 4)

    nc.compile()
    return nc


def _quant(w):
    s = max(float(np.mean(np.abs(w))), EPS_Q)
    return np.clip(np.round(w / s), -1.0, 1.0)


def kernel(x, w1, g1, w2, g2, w3):
    global LAST_RESULTS
    bf = ml_dtypes.bfloat16

    w1q = _quant(np.asarray(w1, np.float32))  # [H, IN]
    w2q = _quant(np.asarray(w2, np.float32))  # [H, H]
    w3q = _quant(np.asarray(w3, np.float32))  # [OUT, H]

    w1t_np = np.zeros([KP, H], dtype=bf)
    w1t_np[:IN] = w1q.T.astype(bf)
    w2t_np = np.ascontiguousarray(w2q.T.astype(bf))
    w3t_np = np.ascontiguousarray(w3q.T.astype(bf))
    g1_np = np.ascontiguousarray(np.asarray(g1, np.float32).reshape(HO, 128).T)
    g2_np = np.ascontiguousarray(np.asarray(g2, np.float32).reshape(HO, 128).T)

    xt_np = np.zeros([KP, B], dtype=bf)
    xt_np[:IN] = np.asarray(x, np.float32).T.astype(bf)

    g_is_one = bool(np.all(np.asarray(g1) == 1.0) and np.all(np.asarray(g2) == 1.0))
    key = ("nc", g_is_one)
    if key not in _cache:
        _cache[key] = _build(g_is_one)
    nc = _cache[key]

    in_maps = []
    for i in range(N_CORES):
        in_maps.append(
            {
                "xt": np.ascontiguousarray(xt_np[:, i * BPC : (i + 1) * BPC]),
                "w1t": w1t_np,
                "w2t": w2t_np,
                "w3t": w3t_np,
                "g1": g1_np,
                "g2": g2_np,
            }
        )

    res = run_bass_kernel_spmd(nc, in_maps, core_ids=list(range(N_CORES)))
    LAST_RESULTS = res

    out = np.empty([B, OUT], dtype=np.float32)
    for i in range(N_CORES):
        out[i * BPC : (i + 1) * BPC] = res.results[i]["outt"].T
    return out
